# revision 27
# baseline (speedup 1.0000x reference)
"""GCN encoder (7-layer GCNConv) on 8 Trainium2 NeuronCores.

Strategy (node-sharded, SPMD), v2 — single-phase-per-layer pipeline:
  - Nodes are permuted and balanced into 8 cores x 10 target-groups of 128
    slots each (degree-balanced bins so every group has <= 2048 unique
    incoming sources = 16 edge-tiles of 128; duplicate sources within a
    group are folded into one slot with the S matrix accumulating norms).
  - Layer pipeline: phase l gathers full-width messages (2KB rows) for its
    target groups from the allgathered z_l in DRAM, segment-sums them on
    the TensorEngine with the sparse indicator S, applies bias+ReLU, and
    as each 256-node chunk of h_l completes it immediately runs the next
    dense transform z_{l+1} = h_l @ W_{l+1} and fires that chunk's
    AllGather, so collectives and dense matmuls hide under the gather DMA
    of the current phase.  Only the last chunk's AllGather is exposed.
  - zf layout is chunk-major ([chunk(5) x core(8) x 256 rows]) so chunked
    AllGather outputs land contiguously; gather indices are precomputed in
    this layout and sorted ascending per group for HBM locality.
  - gcn_norm / bucketing / dedup / permutation are host-side preprocessing;
    all FLOPs (dense transforms + message aggregation) run on device.
"""

import os
import sys
import types

sys.path.insert(0, "/opt/trn_rl_repo")

import numpy as np
import ml_dtypes

NCORES = 8
N = 10000
E = 160000
DIN = 128
DH = 1000
DOUT = 256

TPC = 10  # target groups per core
NP_ = TPC * 128  # 1280 node slots per core
NTOT = NCORES * NP_  # 10240
KT = 16  # edge (unique-source) tiles per group
EPG = KT * 128  # 2048 unique-source slots per group
NLP = 1024  # padded hidden width
NQ = 4  # SWDGE queues
NCH = 5  # node chunks per core (2 groups = 256 nodes each)

BF = ml_dtypes.bfloat16

# conv layer l (1..7) widths: z_l = h_{l-1} @ W_l
LAYER_NL = [NLP] * 6 + [DOUT]  # output width of W_1..W_7
LAYER_KL = [1] + [8] * 6  # k-tiles of W_1..W_7

_CACHE = {}

LAST_EXEC_NS = None
LAST_TRACE = None


def _install_ntff_shim():
    try:
        import antenv

        if hasattr(antenv, "axon_hooks"):
            return
        from trn_agent_boot.trn_boot import _ntff_profile_via_ctypes

        hook = _ntff_profile_via_ctypes("/opt/axon/libaxon_pjrt.so")
        mod = types.ModuleType("antenv.axon_hooks")
        mod.get_axon_ntff_profile_hook = lambda: hook
        mod.set_axon_ntff_profile_hook = lambda h: None
        sys.modules["antenv.axon_hooks"] = mod
        antenv.axon_hooks = mod
    except Exception:
        pass


def _wrap_idx(idx):
    """[n] int -> [128, n/16] int16 (i -> row i%16, col i//16), 8x replicated."""
    n = idx.shape[0]
    w = np.asarray(idx, np.int16).reshape(n // 16, 16).T
    return np.tile(w, (8, 1))


def _build_bass():
    import concourse.mybir as mybir
    from concourse import bacc, tile

    f32 = mybir.dt.float32
    b16 = mybir.dt.bfloat16
    i16 = mybir.dt.int16
    RG = [list(range(NCORES))]

    nc = bacc.Bacc(
        "TRN2",
        target_bir_lowering=False,
        debug=False,
        num_devices=NCORES,
        num_swdge_queues=NQ,
    )

    xz_d = nc.dram_tensor("xz", [NTOT, DIN], b16, kind="ExternalInput").ap()
    w_d = [
        nc.dram_tensor(
            f"w{l}", [128, LAYER_KL[l], LAYER_NL[l]], b16, kind="ExternalInput"
        ).ap()
        for l in range(7)
    ]
    bias_d = [
        nc.dram_tensor(f"bias{l}", [128, LAYER_NL[l]], b16, kind="ExternalInput").ap()
        for l in range(7)
    ]
    ones_d = nc.dram_tensor("ones", [128, 128], b16, kind="ExternalInput").ap()
    s_d = nc.dram_tensor("s", [128, TPC, KT, 128], b16, kind="ExternalInput").ap()
    eidx_d = nc.dram_tensor(
        "eidx", [128, TPC, EPG // 16], i16, kind="ExternalInput"
    ).ap()
    tidx_d = nc.dram_tensor("tidx", [128, 16], i16, kind="ExternalInput").ap()
    out_d = nc.dram_tensor("out", [NP_, DOUT], f32, kind="ExternalOutput").ap()
    out_v = out_d.rearrange("(g p) f -> p g f", p=128)

    qctr = [0]

    def next_q():
        q = qctr[0] % NQ
        qctr[0] += 1
        return q

    with tile.TileContext(nc) as tc:
        with (
            tc.tile_pool(name="const", bufs=1) as cpool,
            tc.tile_pool(name="w", bufs=1) as wpool,
            tc.tile_pool(name="h", bufs=2) as hpool,
            tc.tile_pool(name="ht", bufs=2) as htpool,
            tc.tile_pool(name="z", bufs=3) as zpool,
            tc.tile_pool(name="m", bufs=12) as mpool,
            tc.tile_pool(name="o", bufs=2) as opool,
            tc.tile_pool(name="psD", bufs=2, space="PSUM") as psD,
            tc.tile_pool(name="psA", bufs=6, space="PSUM") as psA,
            tc.tile_pool(name="dram", bufs=3, space="DRAM") as dpool,
        ):
            eidx_sb = cpool.tile([128, TPC, EPG // 16], i16)
            nc.sync.dma_start(eidx_sb[:], eidx_d[:])
            tidx_sb = cpool.tile([128, 16], i16)
            nc.sync.dma_start(tidx_sb[:], tidx_d[:])
            ones_sb = cpool.tile([128, 128], b16)
            nc.sync.dma_start(ones_sb[:], ones_d[:])
            s_sb = cpool.tile([128, TPC, KT, 128], b16)
            nc.sync.dma_start(s_sb[:], s_d[:])
            bias_sb = []
            for l in range(7):
                b_sb = cpool.tile([128, LAYER_NL[l]], b16, name=f"bias_sb{l}")
                nc.sync.dma_start(b_sb[:], bias_d[l][:])
                bias_sb.append(b_sb)

            w_sb = [None] * 7
            w_sb[0] = cpool.tile([128, 1, NLP], b16, name="w_sb0")
            nc.sync.dma_start(w_sb[0][:], w_d[0][:])
            w_sb[1] = wpool.tile([128, 8, NLP], b16, tag="w", name="w_sb1")
            nc.sync.dma_start(w_sb[1][:], w_d[1][:])

            aggx_c = [
                cpool.tile([128, 2, DIN], b16, name=f"aggx_c{ci}") for ci in range(NCH)
            ]
            aggxT_c = [
                cpool.tile([128, 1, 256], b16, name=f"aggxT_c{ci}")
                for ci in range(NCH)
            ]

            h_c = [None] * NCH
            hT_c = [None] * NCH
            hT_c_at = {0: []}  # captured hT APs pending dense, per phase

            def gather_group(src_ap, g, width, phase, nidx=512):
                """Gather the 2048 unique-source rows of group g (full width)
                in nidx-sized sub-tiles so the PE gets work at a fine grain
                (keeps HAM-throttle gaps under the 3.4us window)."""
                tiles = []
                kpc = nidx // 128  # k-tiles per call
                for c in range(EPG // nidx):
                    mt = mpool.tile(
                        [128, kpc, width], b16, tag="m",
                        name=f"msgs{phase}_{g}_{c}",
                    )
                    nc.gpsimd.dma_gather(
                        mt[:],
                        src_ap,
                        eidx_sb[:, g, c * (nidx // 16) : (c + 1) * (nidx // 16)],
                        num_idxs=nidx,
                        num_idxs_reg=nidx,
                        elem_size=width,
                        elem_step=width,
                        queue_num=next_q(),
                    )
                    tiles.append(mt)
                return tiles, kpc

            def agg_matmuls(ap_list, tiles, kpc, g, width, bias, phase):
                """S segment-sum + bias for one group into PSUM tiles
                ap_list[j] covering columns [j*512, ...)."""
                nj = len(ap_list)
                for k in range(KT):
                    mt = tiles[k // kpc]
                    kk = k % kpc
                    for j in range(nj):
                        c0 = j * 512
                        cw = min(512, width - c0)
                        nc.tensor.matmul(
                            ap_list[j][:, 0:cw],
                            s_sb[:, g, k, :],
                            mt[:, kk, c0 : c0 + cw],
                            start=(k == 0),
                            stop=False,
                        )
                for j in range(nj):
                    c0 = j * 512
                    cw = min(512, width - c0)
                    nc.tensor.matmul(
                        ap_list[j][:, 0:cw],
                        ones_sb[:],
                        bias[:, c0 : c0 + cw],
                        start=False,
                        stop=True,
                    )

            def dense_chunk(ci, hT_src, wl, KL, NL, zb, phase):
                """z = hT_chunk @ W for the chunk's two m-tiles; write bf16
                z rows [128, NL] to the bounce zb."""
                for mi in range(2):
                    m = 2 * ci + mi
                    z_sb = zpool.tile([128, NLP], b16, tag="z", name=f"z{phase}_{m}")
                    for n in range((NL + 511) // 512):
                        cw = min(512, NL - n * 512)
                        zp = psD.tile(
                            [128, 512], f32, tag="psD", name=f"zp{phase}_{m}_{n}"
                        )
                        for k in range(KL):
                            nc.tensor.matmul(
                                zp[:, 0:cw],
                                hT_src[:, k, mi * 128 : mi * 128 + 128],
                                wl[:, k, n * 512 : n * 512 + cw],
                                start=(k == 0),
                                stop=(k == KL - 1),
                            )
                        nc.vector.tensor_copy(
                            z_sb[:, n * 512 : n * 512 + cw], zp[:, 0:cw]
                        )
                    nc.sync.dma_start(
                        zb[m * 128 : (m + 1) * 128, 0:NL], z_sb[:, 0:NL]
                    )

            # ---- phase 0: aggregate x, h1 = relu((A x) @ W1 + b1),
            #      z2 = h1 @ W2, chunked AllGather of z2 ----
            zb2 = dpool.tile([NP_, NLP], b16, tag="zb", name="zb2")
            zf2 = dpool.tile(
                [NTOT, NLP], b16, addr_space="Shared", tag="zf", name="zf2"
            )
            def h1_dense(ci):
                # h1 = relu(aggxT @ W1 + b1) for chunk ci, then hT transpose
                h_c[ci] = hpool.tile(
                    [128, 2, NLP], b16, tag="h", name=f"h1_c{ci}"
                )
                hT_c[ci] = htpool.tile(
                    [128, 8, 256], b16, tag="ht", name=f"hT1_c{ci}"
                )
                for mi in range(2):
                    for n in range(2):
                        zp = psD.tile(
                            [128, 512], f32, tag="psD", name=f"h1p_{ci}_{mi}_{n}"
                        )
                        nc.tensor.matmul(
                            zp[:],
                            aggxT_c[ci][:, 0, mi * 128 : mi * 128 + 128],
                            w_sb[0][:, 0, n * 512 : n * 512 + 512],
                            start=True,
                            stop=False,
                        )
                        nc.tensor.matmul(
                            zp[:],
                            ones_sb[:],
                            bias_sb[0][:, n * 512 : n * 512 + 512],
                            start=False,
                            stop=True,
                        )
                        nc.scalar.activation(
                            h_c[ci][:, mi, n * 512 : n * 512 + 512],
                            zp[:],
                            mybir.ActivationFunctionType.Relu,
                        )
                nc.gpsimd.dma_gather(
                    hT_c[ci][:],
                    h_c[ci][:],
                    tidx_sb[:],
                    num_idxs=256,
                    num_idxs_reg=256,
                    elem_size=NLP,
                    transpose=True,
                    sbuf_tokens_per_rank=128,
                    sbuf_free_dim_per_rank=NLP * 2,
                    queue_num=next_q(),
                )

            pend_a = []  # chunks with aggxT issued, h1 dense not yet run
            pend_b = []  # chunks with h1/hT done, z2 dense not yet run
            for g in range(TPC):
                tiles, kpc0 = gather_group(xz_d[:], g, DIN, 0, nidx=1024)
                if pend_b:
                    dense_chunk(pend_b.pop(0), hT_c_at[0].pop(0), w_sb[1][:],
                                8, NLP, zb2[:], 0)
                if pend_a:
                    ci0 = pend_a.pop(0)
                    h1_dense(ci0)
                    pend_b.append(ci0)
                    hT_c_at[0].append(hT_c[ci0][:])
                ap0 = psA.tile([128, 512], f32, tag="psA", name=f"ap0_{g}")
                for k in range(KT):
                    mt = tiles[k // kpc0]
                    nc.tensor.matmul(
                        ap0[:, 0:DIN],
                        s_sb[:, g, k, :],
                        mt[:, k % kpc0, :],
                        start=(k == 0),
                        stop=(k == KT - 1),
                    )
                nc.scalar.activation(
                    aggx_c[g // 2][:, g % 2, :],
                    ap0[:, 0:DIN],
                    mybir.ActivationFunctionType.Copy,
                )
                if g % 2 == 1:
                    ci = g // 2
                    nc.gpsimd.dma_gather(
                        aggxT_c[ci][:],
                        aggx_c[ci][:],
                        tidx_sb[:],
                        num_idxs=256,
                        num_idxs_reg=256,
                        elem_size=DIN,
                        transpose=True,
                        sbuf_tokens_per_rank=128,
                        sbuf_free_dim_per_rank=DIN * 2,
                        queue_num=next_q(),
                    )
                    pend_a.append(ci)
            while pend_a or pend_b:
                if pend_b:
                    dense_chunk(pend_b.pop(0), hT_c_at[0].pop(0), w_sb[1][:],
                                8, NLP, zb2[:], 0)
                if pend_a:
                    ci0 = pend_a.pop(0)
                    h1_dense(ci0)
                    pend_b.append(ci0)
                    hT_c_at[0].append(hT_c[ci0][:])
            nc.gpsimd.collective_compute(
                "AllGather",
                mybir.AluOpType.bypass,
                replica_groups=RG,
                ins=[zb2[:].opt()],
                outs=[zf2[:].opt()],
            )

            # ---- phases 2..7: gather z_l messages, aggregate, and (for
            #      l<7) run next dense + chunked AllGather in-phase ----
            zf_prev = zf2
            for l in range(2, 8):
                NL = LAYER_NL[l - 1]  # width of z_l
                if l < 7:
                    NLn = LAYER_NL[l]
                    w_sb[l] = wpool.tile(
                        [128, LAYER_KL[l], LAYER_NL[l]], b16, tag="w",
                        name=f"w_sb{l}",
                    )
                    nc.sync.dma_start(w_sb[l][:], w_d[l][:])
                    zb_n = dpool.tile([NP_, NLn], b16, tag="zb", name=f"zb{l + 1}")
                    zf_n = dpool.tile(
                        [NTOT, NLn], b16, addr_space="Shared", tag="zf",
                        name=f"zf{l + 1}",
                    )
                pend = []  # (ci, hT ap) with hT issued, dense not yet run
                for g in range(TPC):
                    tiles, kpc = gather_group(
                        zf_prev[:], g, NL, l, nidx=(512 if l < 7 else 1024)
                    )
                    if pend:
                        ci0, hT0 = pend.pop(0)
                        dense_chunk(ci0, hT0, w_sb[l][:], LAYER_KL[l], NLn,
                                    zb_n[:], l)
                    naps = (NL + 511) // 512
                    aps = [
                        psA.tile(
                            [128, 512], f32, tag="psA", name=f"ap{l}_{g}_{j}"
                        )
                        for j in range(naps)
                    ]
                    agg_matmuls(aps, tiles, kpc, g, NL, bias_sb[l - 1][:], l)
                    if l < 7:
                        ci = g // 2
                        if g % 2 == 0:
                            h_c[ci] = hpool.tile(
                                [128, 2, NLP], b16, tag="h", name=f"h{l}_c{ci}"
                            )
                            hT_c[ci] = htpool.tile(
                                [128, 8, 256], b16, tag="ht",
                                name=f"hT{l}_c{ci}",
                            )
                        for j in range(naps):
                            nc.scalar.activation(
                                h_c[ci][:, g % 2, j * 512 : j * 512 + 512],
                                aps[j][:],
                                mybir.ActivationFunctionType.Relu,
                            )
                        if g % 2 == 1:
                            nc.gpsimd.dma_gather(
                                hT_c[ci][:],
                                h_c[ci][:],
                                tidx_sb[:],
                                num_idxs=256,
                                num_idxs_reg=256,
                                elem_size=NLP,
                                transpose=True,
                                sbuf_tokens_per_rank=128,
                                sbuf_free_dim_per_rank=NLP * 2,
                                queue_num=next_q(),
                            )
                            pend.append((ci, hT_c[ci][:]))
                    else:
                        o_sb = opool.tile([128, DOUT], f32, tag="o", name=f"o{g}")
                        nc.scalar.activation(
                            o_sb[:], aps[0][:, 0:DOUT],
                            mybir.ActivationFunctionType.Copy,
                        )
                        nc.sync.dma_start(out_v[:, g, :], o_sb[:])
                if l < 7:
                    while pend:
                        ci0, hT0 = pend.pop(0)
                        dense_chunk(ci0, hT0, w_sb[l][:], LAYER_KL[l], NLn,
                                    zb_n[:], l)
                    nc.gpsimd.collective_compute(
                        "AllGather",
                        mybir.AluOpType.bypass,
                        replica_groups=RG,
                        ins=[zb_n[:].opt()],
                        outs=[zf_n[:].opt()],
                    )
                    zf_prev = zf_n

    # Align each gather's SWDGE queue with its Tile-assigned DMASW sem lane
    # (ucode locks each DMA sem to one queue; Tile assigns lanes round-robin
    # in scheduled order, so queue must be derived from the lane, not vice
    # versa).
    from concourse.tile_sem_assignment import PROC_NAME_TO_IDX

    lane_to_q = {PROC_NAME_TO_IDX[f"DMASW{i}"]: i % NQ for i in range(8)}
    for bb in nc.main_func.blocks:
        for inst in bb.instructions:
            if isinstance(inst, mybir.InstDMAGatherAnt):
                proc = getattr(inst, "bass_scheduled_proc", None)
                if proc in lane_to_q:
                    inst.queue_num = lane_to_q[proc]

    nc.compile()
    return nc


def _preprocess(x, edge_index, edge_weight):
    """gcn_norm + node permutation + per-group source dedup (host side)."""
    ei = np.asarray(edge_index)
    row = np.concatenate([ei[0], np.arange(N)]).astype(np.int64)
    col = np.concatenate([ei[1], np.arange(N)]).astype(np.int64)
    w = np.concatenate(
        [np.asarray(edge_weight, np.float64), np.ones(N, np.float64)]
    )
    deg = np.zeros(N, np.float64)
    np.add.at(deg, col, w)
    dis = np.where(deg > 0, 1.0 / np.sqrt(deg), 0.0)
    norm = (dis[row] * w * dis[col]).astype(np.float32)

    # balance nodes into 80 bins (cap 128 nodes) by in-degree
    indeg = np.bincount(col, minlength=N)
    NB = NCORES * TPC
    order = np.argsort(-indeg, kind="stable")
    load = np.zeros(NB, np.int64)
    cnt = np.zeros(NB, np.int64)
    binof = np.empty(N, np.int64)
    slotof = np.empty(N, np.int64)
    for v in order:
        feas = np.flatnonzero(cnt < 128)
        b = feas[np.argmin(load[feas])]
        binof[v] = b
        slotof[v] = cnt[b]
        cnt[b] += 1
        load[b] += indeg[v]
    core = binof // TPC
    grp = binof % TPC
    pid = core * NP_ + grp * 128 + slotof  # permuted global id (output order)
    # zf row id: rank-major allgather layout == pid order
    zfrow = pid

    # per-bin unique sources (sorted by zf row for gather locality)
    ebin = binof[col]
    S = np.zeros((NCORES, 128, TPC, KT, 128), np.float32)
    IDX = np.zeros((NCORES, TPC, EPG), np.int64)
    esrc_zf = zfrow[row]
    etslot = slotof[col]
    order_e = np.argsort(ebin, kind="stable")
    ebin_s = ebin[order_e]
    bounds = np.searchsorted(ebin_s, np.arange(NB + 1))
    for b in range(NB):
        sel = order_e[bounds[b] : bounds[b + 1]]
        srcs = esrc_zf[sel]
        tgts = etslot[sel]
        nms = norm[sel]
        uniq, inv = np.unique(srcs, return_inverse=True)
        u = len(uniq)
        assert u <= EPG, f"bin {b}: {u} unique sources > {EPG}"
        c, g = b // TPC, b % TPC
        IDX[c, g, :u] = uniq
        np.add.at(S[c], (inv % 128, g, inv // 128, tgts), nms)
    return pid, S, IDX


def kernel(x, edge_index, edge_weight, W1, b1, Wmid, bmid, W7, b7):
    global LAST_EXEC_NS, LAST_TRACE
    trace = os.environ.get("GCN_TRACE") == "1"
    if trace:
        _install_ntff_shim()

    from concourse import bass_utils

    x = np.asarray(x, np.float32)
    pid, S, IDX = _preprocess(x, edge_index, edge_weight)

    # x in zf-row (= pid) order, bf16, empty slots zero; replicated per core
    xz = np.zeros((NTOT, DIN), np.float32)
    xz[pid] = x
    xz = xz.astype(BF)

    # weights / biases, padded + k-striped, bf16
    def kstripe(W, KL, NL):
        Wp = np.zeros((KL * 128, NL), np.float32)
        Wp[: W.shape[0], : W.shape[1]] = np.asarray(W, np.float32)
        return Wp.reshape(KL, 128, NL).transpose(1, 0, 2).astype(BF)

    Ws = [kstripe(np.asarray(W1), 1, NLP)]
    for i in range(5):
        Ws.append(kstripe(np.asarray(Wmid)[i], 8, NLP))
    Ws.append(kstripe(np.asarray(W7), 8, DOUT))
    bs = []
    for i, b in enumerate([b1] + [np.asarray(bmid)[i] for i in range(5)] + [b7]):
        NL = LAYER_NL[i]
        bp = np.zeros(NL, np.float32)
        bp[: b.shape[0]] = np.asarray(b, np.float32)
        bs.append(np.broadcast_to(bp.astype(BF), (128, NL)).copy())

    ones = np.full((128, 128), 1.0 / 128.0, np.float32).astype(BF)
    tidx = _wrap_idx(np.arange(256))

    if "nc" not in _CACHE:
        _CACHE["nc"] = _build_bass()
    nc = _CACHE["nc"]

    in_maps = []
    for c in range(NCORES):
        eidx_c = np.stack(
            [_wrap_idx(IDX[c, g]) for g in range(TPC)], axis=1
        )  # [128, TPC, EPG/16]
        m = {
            "xz": xz,
            "ones": ones,
            "s": np.ascontiguousarray(S[c].astype(BF)),
            "eidx": np.ascontiguousarray(eidx_c),
            "tidx": tidx,
        }
        for l in range(7):
            m[f"w{l}"] = Ws[l]
            m[f"bias{l}"] = bs[l]
        in_maps.append(m)

    res = bass_utils.run_bass_kernel_spmd(
        nc, in_maps, core_ids=list(range(NCORES)), trace=trace
    )
    if trace:
        LAST_EXEC_NS = res.exec_time_ns
        LAST_TRACE = res.profile_json
        print(f"HW exec time: {res.exec_time_ns} ns")
        if res.instructions_and_trace is not None:
            print(f"trace: {res.instructions_and_trace[1]}")

    percore = np.stack([res.results[c]["out"] for c in range(NCORES)])  # [8,1280,256]
    out_full = percore[pid // NP_, pid % NP_]
    return out_full


# revision 28
# speedup vs baseline: 1.0246x; 1.0246x over previous
"""GCN encoder (7-layer GCNConv) on 8 Trainium2 NeuronCores.

Strategy (node-sharded, SPMD), v2 — single-phase-per-layer pipeline:
  - Nodes are permuted and balanced into 8 cores x 10 target-groups of 128
    slots each (degree-balanced bins so every group has <= 2048 unique
    incoming sources = 16 edge-tiles of 128; duplicate sources within a
    group are folded into one slot with the S matrix accumulating norms).
  - Layer pipeline: phase l gathers full-width messages (2KB rows) for its
    target groups from the allgathered z_l in DRAM, segment-sums them on
    the TensorEngine with the sparse indicator S, applies bias+ReLU, and
    as each 256-node chunk of h_l completes it immediately runs the next
    dense transform z_{l+1} = h_l @ W_{l+1} and fires that chunk's
    AllGather, so collectives and dense matmuls hide under the gather DMA
    of the current phase.  Only the last chunk's AllGather is exposed.
  - zf layout is chunk-major ([chunk(5) x core(8) x 256 rows]) so chunked
    AllGather outputs land contiguously; gather indices are precomputed in
    this layout and sorted ascending per group for HBM locality.
  - gcn_norm / bucketing / dedup / permutation are host-side preprocessing;
    all FLOPs (dense transforms + message aggregation) run on device.
"""

import os
import sys
import types

sys.path.insert(0, "/opt/trn_rl_repo")

import numpy as np
import ml_dtypes

NCORES = 8
N = 10000
E = 160000
DIN = 128
DH = 1000
DOUT = 256

TPC = 10  # target groups per core
NP_ = TPC * 128  # 1280 node slots per core
NTOT = NCORES * NP_  # 10240
KT = 16  # edge (unique-source) tiles per group
EPG = KT * 128  # 2048 unique-source slots per group
NLP = 1024  # padded hidden width
NQ = 4  # SWDGE queues
NCH = 5  # node chunks per core (2 groups = 256 nodes each)

BF = ml_dtypes.bfloat16

# conv layer l (1..7) widths: z_l = h_{l-1} @ W_l
LAYER_NL = [NLP] * 6 + [DOUT]  # output width of W_1..W_7
LAYER_KL = [1] + [8] * 6  # k-tiles of W_1..W_7

_CACHE = {}

LAST_EXEC_NS = None
LAST_TRACE = None


def _install_ntff_shim():
    try:
        import antenv

        if hasattr(antenv, "axon_hooks"):
            return
        from trn_agent_boot.trn_boot import _ntff_profile_via_ctypes

        hook = _ntff_profile_via_ctypes("/opt/axon/libaxon_pjrt.so")
        mod = types.ModuleType("antenv.axon_hooks")
        mod.get_axon_ntff_profile_hook = lambda: hook
        mod.set_axon_ntff_profile_hook = lambda h: None
        sys.modules["antenv.axon_hooks"] = mod
        antenv.axon_hooks = mod
    except Exception:
        pass


def _wrap_idx(idx):
    """[n] int -> [128, n/16] int16 (i -> row i%16, col i//16), 8x replicated."""
    n = idx.shape[0]
    w = np.asarray(idx, np.int16).reshape(n // 16, 16).T
    return np.tile(w, (8, 1))


def _build_bass():
    import concourse.mybir as mybir
    from concourse import bacc, tile

    f32 = mybir.dt.float32
    b16 = mybir.dt.bfloat16
    i16 = mybir.dt.int16
    RG = [list(range(NCORES))]

    nc = bacc.Bacc(
        "TRN2",
        target_bir_lowering=False,
        debug=False,
        num_devices=NCORES,
        num_swdge_queues=NQ,
    )

    xz_d = nc.dram_tensor("xz", [NTOT, DIN], b16, kind="ExternalInput").ap()
    w_d = [
        nc.dram_tensor(
            f"w{l}", [128, LAYER_KL[l], LAYER_NL[l]], b16, kind="ExternalInput"
        ).ap()
        for l in range(7)
    ]
    bias_d = [
        nc.dram_tensor(f"bias{l}", [128, LAYER_NL[l]], b16, kind="ExternalInput").ap()
        for l in range(7)
    ]
    ones_d = nc.dram_tensor("ones", [128, 128], b16, kind="ExternalInput").ap()
    s_d = nc.dram_tensor("s", [128, TPC, KT, 128], b16, kind="ExternalInput").ap()
    eidx_d = nc.dram_tensor(
        "eidx", [128, TPC, EPG // 16], i16, kind="ExternalInput"
    ).ap()
    tidx_d = nc.dram_tensor("tidx", [128, 16], i16, kind="ExternalInput").ap()
    out_d = nc.dram_tensor("out", [NP_, DOUT], f32, kind="ExternalOutput").ap()
    out_v = out_d.rearrange("(g p) f -> p g f", p=128)

    qctr = [0]

    def next_q():
        q = qctr[0] % NQ
        qctr[0] += 1
        return q

    with tile.TileContext(nc) as tc:
        with (
            tc.tile_pool(name="const", bufs=1) as cpool,
            tc.tile_pool(name="w", bufs=1) as wpool,
            tc.tile_pool(name="h", bufs=2) as hpool,
            tc.tile_pool(name="ht", bufs=2) as htpool,
            tc.tile_pool(name="z", bufs=3) as zpool,
            tc.tile_pool(name="m", bufs=11) as mpool,
            tc.tile_pool(name="o", bufs=2) as opool,
            tc.tile_pool(name="psD", bufs=2, space="PSUM") as psD,
            tc.tile_pool(name="psA", bufs=6, space="PSUM") as psA,
            tc.tile_pool(name="dram", bufs=3, space="DRAM") as dpool,
        ):
            eidx_sb = cpool.tile([128, TPC, EPG // 16], i16)
            nc.sync.dma_start(eidx_sb[:], eidx_d[:])
            tidx_sb = cpool.tile([128, 16], i16)
            nc.sync.dma_start(tidx_sb[:], tidx_d[:])
            ones_sb = cpool.tile([128, 128], b16)
            nc.sync.dma_start(ones_sb[:], ones_d[:])
            s_sb = cpool.tile([128, TPC, KT, 128], b16)
            nc.sync.dma_start(s_sb[:], s_d[:])
            bias_sb = []
            for l in range(7):
                b_sb = cpool.tile([128, LAYER_NL[l]], b16, name=f"bias_sb{l}")
                nc.sync.dma_start(b_sb[:], bias_d[l][:])
                bias_sb.append(b_sb)

            w_sb = [None] * 7
            w_sb[0] = cpool.tile([128, 1, NLP], b16, name="w_sb0")
            nc.sync.dma_start(w_sb[0][:], w_d[0][:])
            w_sb[1] = wpool.tile([128, 8, NLP], b16, tag="w", name="w_sb1")
            nc.sync.dma_start(w_sb[1][:], w_d[1][:])

            aggx_c = [
                cpool.tile([128, 2, DIN], b16, name=f"aggx_c{ci}") for ci in range(NCH)
            ]
            aggxT_c = [
                cpool.tile([128, 1, 256], b16, name=f"aggxT_c{ci}")
                for ci in range(NCH)
            ]

            h_c = [None] * NCH
            hT_c = [None] * NCH
            hT_c_at = {0: []}  # captured hT APs pending dense, per phase

            def gather_group(src_ap, g, width, phase, nidx=512):
                """Gather the 2048 unique-source rows of group g (full width)
                in nidx-sized sub-tiles so the PE gets work at a fine grain
                (keeps HAM-throttle gaps under the 3.4us window)."""
                tiles = []
                kpc = nidx // 128  # k-tiles per call
                for c in range(EPG // nidx):
                    mt = mpool.tile(
                        [128, kpc, width], b16, tag="m",
                        name=f"msgs{phase}_{g}_{c}",
                    )
                    nc.gpsimd.dma_gather(
                        mt[:],
                        src_ap,
                        eidx_sb[:, g, c * (nidx // 16) : (c + 1) * (nidx // 16)],
                        num_idxs=nidx,
                        num_idxs_reg=nidx,
                        elem_size=width,
                        elem_step=width,
                        queue_num=next_q(),
                    )
                    tiles.append(mt)
                return tiles, kpc

            def agg_matmuls(ap_list, tiles, kpc, g, width, bias, phase):
                """S segment-sum + bias for one group into PSUM tiles
                ap_list[j] covering columns [j*512, ...)."""
                nj = len(ap_list)
                for k in range(KT):
                    mt = tiles[k // kpc]
                    kk = k % kpc
                    for j in range(nj):
                        c0 = j * 512
                        cw = min(512, width - c0)
                        nc.tensor.matmul(
                            ap_list[j][:, 0:cw],
                            s_sb[:, g, k, :],
                            mt[:, kk, c0 : c0 + cw],
                            start=(k == 0),
                            stop=False,
                        )
                for j in range(nj):
                    c0 = j * 512
                    cw = min(512, width - c0)
                    nc.tensor.matmul(
                        ap_list[j][:, 0:cw],
                        ones_sb[:],
                        bias[:, c0 : c0 + cw],
                        start=False,
                        stop=True,
                    )

            def dense_chunk(ci, hT_src, wl, KL, NL, zb, phase):
                """z = hT_chunk @ W for the chunk's two m-tiles; write bf16
                z rows [128, NL] to the bounce zb."""
                for mi in range(2):
                    m = 2 * ci + mi
                    z_sb = zpool.tile([128, NLP], b16, tag="z", name=f"z{phase}_{m}")
                    for n in range((NL + 511) // 512):
                        cw = min(512, NL - n * 512)
                        zp = psD.tile(
                            [128, 512], f32, tag="psD", name=f"zp{phase}_{m}_{n}"
                        )
                        for k in range(KL):
                            nc.tensor.matmul(
                                zp[:, 0:cw],
                                hT_src[:, k, mi * 128 : mi * 128 + 128],
                                wl[:, k, n * 512 : n * 512 + cw],
                                start=(k == 0),
                                stop=(k == KL - 1),
                            )
                        nc.vector.tensor_copy(
                            z_sb[:, n * 512 : n * 512 + cw], zp[:, 0:cw]
                        )
                    nc.sync.dma_start(
                        zb[m * 128 : (m + 1) * 128, 0:NL], z_sb[:, 0:NL]
                    )

            # ---- phase 0: aggregate x, h1 = relu((A x) @ W1 + b1),
            #      z2 = h1 @ W2, chunked AllGather of z2 ----
            zb2 = dpool.tile([NP_, NLP], b16, tag="zb", name="zb2")
            zf2 = dpool.tile(
                [NTOT, NLP], b16, addr_space="Shared", tag="zf", name="zf2"
            )
            def h1_dense(ci):
                # h1 = relu(aggxT @ W1 + b1) for chunk ci, then hT transpose
                h_c[ci] = hpool.tile(
                    [128, 2, NLP], b16, tag="h", name=f"h1_c{ci}"
                )
                hT_c[ci] = htpool.tile(
                    [128, 8, 256], b16, tag="ht", name=f"hT1_c{ci}"
                )
                for mi in range(2):
                    for n in range(2):
                        zp = psD.tile(
                            [128, 512], f32, tag="psD", name=f"h1p_{ci}_{mi}_{n}"
                        )
                        nc.tensor.matmul(
                            zp[:],
                            aggxT_c[ci][:, 0, mi * 128 : mi * 128 + 128],
                            w_sb[0][:, 0, n * 512 : n * 512 + 512],
                            start=True,
                            stop=False,
                        )
                        nc.tensor.matmul(
                            zp[:],
                            ones_sb[:],
                            bias_sb[0][:, n * 512 : n * 512 + 512],
                            start=False,
                            stop=True,
                        )
                        nc.scalar.activation(
                            h_c[ci][:, mi, n * 512 : n * 512 + 512],
                            zp[:],
                            mybir.ActivationFunctionType.Relu,
                        )
                nc.gpsimd.dma_gather(
                    hT_c[ci][:],
                    h_c[ci][:],
                    tidx_sb[:],
                    num_idxs=256,
                    num_idxs_reg=256,
                    elem_size=NLP,
                    transpose=True,
                    sbuf_tokens_per_rank=128,
                    sbuf_free_dim_per_rank=NLP * 2,
                    queue_num=next_q(),
                )

            pend_a = []  # chunks with aggxT issued, h1 dense not yet run
            pend_b = []  # chunks with h1/hT done, z2 dense not yet run
            for g in range(TPC):
                tiles, kpc0 = gather_group(xz_d[:], g, DIN, 0, nidx=1024)
                if pend_b:
                    dense_chunk(pend_b.pop(0), hT_c_at[0].pop(0), w_sb[1][:],
                                8, NLP, zb2[:], 0)
                if pend_a:
                    ci0 = pend_a.pop(0)
                    h1_dense(ci0)
                    pend_b.append(ci0)
                    hT_c_at[0].append(hT_c[ci0][:])
                ap0 = psA.tile([128, 512], f32, tag="psA", name=f"ap0_{g}")
                for k in range(KT):
                    mt = tiles[k // kpc0]
                    nc.tensor.matmul(
                        ap0[:, 0:DIN],
                        s_sb[:, g, k, :],
                        mt[:, k % kpc0, :],
                        start=(k == 0),
                        stop=(k == KT - 1),
                    )
                nc.scalar.activation(
                    aggx_c[g // 2][:, g % 2, :],
                    ap0[:, 0:DIN],
                    mybir.ActivationFunctionType.Copy,
                )
                if g % 2 == 1:
                    ci = g // 2
                    nc.gpsimd.dma_gather(
                        aggxT_c[ci][:],
                        aggx_c[ci][:],
                        tidx_sb[:],
                        num_idxs=256,
                        num_idxs_reg=256,
                        elem_size=DIN,
                        transpose=True,
                        sbuf_tokens_per_rank=128,
                        sbuf_free_dim_per_rank=DIN * 2,
                        queue_num=next_q(),
                    )
                    pend_a.append(ci)
            while pend_a or pend_b:
                if pend_b:
                    dense_chunk(pend_b.pop(0), hT_c_at[0].pop(0), w_sb[1][:],
                                8, NLP, zb2[:], 0)
                if pend_a:
                    ci0 = pend_a.pop(0)
                    h1_dense(ci0)
                    pend_b.append(ci0)
                    hT_c_at[0].append(hT_c[ci0][:])
            nc.gpsimd.collective_compute(
                "AllGather",
                mybir.AluOpType.bypass,
                replica_groups=RG,
                ins=[zb2[:].opt()],
                outs=[zf2[:].opt()],
            )

            # ---- phases 2..7: gather z_l messages, aggregate, and (for
            #      l<7) run next dense + chunked AllGather in-phase ----
            zf_prev = zf2
            for l in range(2, 8):
                NL = LAYER_NL[l - 1]  # width of z_l
                if l < 7:
                    NLn = LAYER_NL[l]
                    w_sb[l] = wpool.tile(
                        [128, LAYER_KL[l], LAYER_NL[l]], b16, tag="w",
                        name=f"w_sb{l}",
                    )
                    nc.sync.dma_start(w_sb[l][:], w_d[l][:])
                    zb_n = dpool.tile([NP_, NLn], b16, tag="zb", name=f"zb{l + 1}")
                    zf_n = dpool.tile(
                        [NTOT, NLn], b16, addr_space="Shared", tag="zf",
                        name=f"zf{l + 1}",
                    )
                pend = []  # (ci, hT ap) with hT issued, dense not yet run
                for g in range(TPC):
                    tiles, kpc = gather_group(zf_prev[:], g, NL, l)
                    if pend:
                        ci0, hT0 = pend.pop(0)
                        dense_chunk(ci0, hT0, w_sb[l][:], LAYER_KL[l], NLn,
                                    zb_n[:], l)
                    naps = (NL + 511) // 512
                    aps = [
                        psA.tile(
                            [128, 512], f32, tag="psA", name=f"ap{l}_{g}_{j}"
                        )
                        for j in range(naps)
                    ]
                    agg_matmuls(aps, tiles, kpc, g, NL, bias_sb[l - 1][:], l)
                    if l < 7:
                        ci = g // 2
                        if g % 2 == 0:
                            h_c[ci] = hpool.tile(
                                [128, 2, NLP], b16, tag="h", name=f"h{l}_c{ci}"
                            )
                            hT_c[ci] = htpool.tile(
                                [128, 8, 256], b16, tag="ht",
                                name=f"hT{l}_c{ci}",
                            )
                        for j in range(naps):
                            nc.scalar.activation(
                                h_c[ci][:, g % 2, j * 512 : j * 512 + 512],
                                aps[j][:],
                                mybir.ActivationFunctionType.Relu,
                            )
                        if g % 2 == 1:
                            nc.gpsimd.dma_gather(
                                hT_c[ci][:],
                                h_c[ci][:],
                                tidx_sb[:],
                                num_idxs=256,
                                num_idxs_reg=256,
                                elem_size=NLP,
                                transpose=True,
                                sbuf_tokens_per_rank=128,
                                sbuf_free_dim_per_rank=NLP * 2,
                                queue_num=next_q(),
                            )
                            pend.append((ci, hT_c[ci][:]))
                    else:
                        o_sb = opool.tile([128, DOUT], f32, tag="o", name=f"o{g}")
                        nc.scalar.activation(
                            o_sb[:], aps[0][:, 0:DOUT],
                            mybir.ActivationFunctionType.Copy,
                        )
                        nc.sync.dma_start(out_v[:, g, :], o_sb[:])
                if l < 7:
                    while pend:
                        ci0, hT0 = pend.pop(0)
                        dense_chunk(ci0, hT0, w_sb[l][:], LAYER_KL[l], NLn,
                                    zb_n[:], l)
                    nc.gpsimd.collective_compute(
                        "AllGather",
                        mybir.AluOpType.bypass,
                        replica_groups=RG,
                        ins=[zb_n[:].opt()],
                        outs=[zf_n[:].opt()],
                    )
                    zf_prev = zf_n

    # Align each gather's SWDGE queue with its Tile-assigned DMASW sem lane
    # (ucode locks each DMA sem to one queue; Tile assigns lanes round-robin
    # in scheduled order, so queue must be derived from the lane, not vice
    # versa).
    from concourse.tile_sem_assignment import PROC_NAME_TO_IDX

    lane_to_q = {PROC_NAME_TO_IDX[f"DMASW{i}"]: i % NQ for i in range(8)}
    for bb in nc.main_func.blocks:
        for inst in bb.instructions:
            if isinstance(inst, mybir.InstDMAGatherAnt):
                proc = getattr(inst, "bass_scheduled_proc", None)
                if proc in lane_to_q:
                    inst.queue_num = lane_to_q[proc]

    nc.compile()
    return nc


def _preprocess(x, edge_index, edge_weight):
    """gcn_norm + node permutation + per-group source dedup (host side)."""
    ei = np.asarray(edge_index)
    row = np.concatenate([ei[0], np.arange(N)]).astype(np.int64)
    col = np.concatenate([ei[1], np.arange(N)]).astype(np.int64)
    w = np.concatenate(
        [np.asarray(edge_weight, np.float64), np.ones(N, np.float64)]
    )
    deg = np.zeros(N, np.float64)
    np.add.at(deg, col, w)
    dis = np.where(deg > 0, 1.0 / np.sqrt(deg), 0.0)
    norm = (dis[row] * w * dis[col]).astype(np.float32)

    # balance nodes into 80 bins (cap 128 nodes) by in-degree
    indeg = np.bincount(col, minlength=N)
    NB = NCORES * TPC
    order = np.argsort(-indeg, kind="stable")
    load = np.zeros(NB, np.int64)
    cnt = np.zeros(NB, np.int64)
    binof = np.empty(N, np.int64)
    slotof = np.empty(N, np.int64)
    for v in order:
        feas = np.flatnonzero(cnt < 128)
        b = feas[np.argmin(load[feas])]
        binof[v] = b
        slotof[v] = cnt[b]
        cnt[b] += 1
        load[b] += indeg[v]
    core = binof // TPC
    grp = binof % TPC
    pid = core * NP_ + grp * 128 + slotof  # permuted global id (output order)
    # zf row id: rank-major allgather layout == pid order
    zfrow = pid

    # per-bin unique sources (sorted by zf row for gather locality)
    ebin = binof[col]
    S = np.zeros((NCORES, 128, TPC, KT, 128), np.float32)
    IDX = np.zeros((NCORES, TPC, EPG), np.int64)
    esrc_zf = zfrow[row]
    etslot = slotof[col]
    order_e = np.argsort(ebin, kind="stable")
    ebin_s = ebin[order_e]
    bounds = np.searchsorted(ebin_s, np.arange(NB + 1))
    for b in range(NB):
        sel = order_e[bounds[b] : bounds[b + 1]]
        srcs = esrc_zf[sel]
        tgts = etslot[sel]
        nms = norm[sel]
        uniq, inv = np.unique(srcs, return_inverse=True)
        u = len(uniq)
        assert u <= EPG, f"bin {b}: {u} unique sources > {EPG}"
        c, g = b // TPC, b % TPC
        IDX[c, g, :u] = uniq
        np.add.at(S[c], (inv % 128, g, inv // 128, tgts), nms)
    return pid, S, IDX


def kernel(x, edge_index, edge_weight, W1, b1, Wmid, bmid, W7, b7):
    global LAST_EXEC_NS, LAST_TRACE
    trace = os.environ.get("GCN_TRACE") == "1"
    if trace:
        _install_ntff_shim()

    from concourse import bass_utils

    x = np.asarray(x, np.float32)
    pid, S, IDX = _preprocess(x, edge_index, edge_weight)

    # x in zf-row (= pid) order, bf16, empty slots zero; replicated per core
    xz = np.zeros((NTOT, DIN), np.float32)
    xz[pid] = x
    xz = xz.astype(BF)

    # weights / biases, padded + k-striped, bf16
    def kstripe(W, KL, NL):
        Wp = np.zeros((KL * 128, NL), np.float32)
        Wp[: W.shape[0], : W.shape[1]] = np.asarray(W, np.float32)
        return Wp.reshape(KL, 128, NL).transpose(1, 0, 2).astype(BF)

    Ws = [kstripe(np.asarray(W1), 1, NLP)]
    for i in range(5):
        Ws.append(kstripe(np.asarray(Wmid)[i], 8, NLP))
    Ws.append(kstripe(np.asarray(W7), 8, DOUT))
    bs = []
    for i, b in enumerate([b1] + [np.asarray(bmid)[i] for i in range(5)] + [b7]):
        NL = LAYER_NL[i]
        bp = np.zeros(NL, np.float32)
        bp[: b.shape[0]] = np.asarray(b, np.float32)
        bs.append(np.broadcast_to(bp.astype(BF), (128, NL)).copy())

    ones = np.full((128, 128), 1.0 / 128.0, np.float32).astype(BF)
    tidx = _wrap_idx(np.arange(256))

    if "nc" not in _CACHE:
        _CACHE["nc"] = _build_bass()
    nc = _CACHE["nc"]

    in_maps = []
    for c in range(NCORES):
        eidx_c = np.stack(
            [_wrap_idx(IDX[c, g]) for g in range(TPC)], axis=1
        )  # [128, TPC, EPG/16]
        m = {
            "xz": xz,
            "ones": ones,
            "s": np.ascontiguousarray(S[c].astype(BF)),
            "eidx": np.ascontiguousarray(eidx_c),
            "tidx": tidx,
        }
        for l in range(7):
            m[f"w{l}"] = Ws[l]
            m[f"bias{l}"] = bs[l]
        in_maps.append(m)

    res = bass_utils.run_bass_kernel_spmd(
        nc, in_maps, core_ids=list(range(NCORES)), trace=trace
    )
    if trace:
        LAST_EXEC_NS = res.exec_time_ns
        LAST_TRACE = res.profile_json
        print(f"HW exec time: {res.exec_time_ns} ns")
        if res.instructions_and_trace is not None:
            print(f"trace: {res.instructions_and_trace[1]}")

    percore = np.stack([res.results[c]["out"] for c in range(NCORES)])  # [8,1280,256]
    out_full = percore[pid // NP_, pid % NP_]
    return out_full
